# revision 14
# baseline (speedup 1.0000x reference)
"""ChannelTimeAttention Trainium2 kernel (v2 — wide-descriptor DMA).

Reference computation (per (b, c) pair, all independent):
    pooled = AdaptiveAvgPool(x[b, :, c]) -> [t, 8*8]      (7x7 block means)
    q = pooled @ Wq + bq ; k = pooled @ Wk + bk           [t, 32]
    att = softmax(q @ k.T / sqrt(t))                      [t, t]
    out[b, :, c] = att @ x[b, :, c].reshape(t, h*w)

Sharding: data-parallel over b — one batch element per NeuronCore (8 cores).

DMA design: the per-descriptor fixed overhead (~280ns) on 12.5KB
per-partition rows wastes ~35% of DMA-engine bandwidth.  Each SBUF
partition instead holds TWO channels' rows (2*12544B contiguous DRAM), so
descriptors are 25088B (~20 GB/s/engine vs 16.4).  x is loaded in 4 tiles
of [128, 2*3136] (16 channels each, partition (t*8+j) <-> channels
16k+2j+{0,1}); outputs mirror this.  All 4 input DMAs ride the scalar
HWDGE queue back-to-back so tiles arrive in order; outputs alternate
sync/gpsimd queues.  Inputs+outputs share one 6-slot SBUF pool (WAR
semaphores recycle input slots for late output tiles).

Compute per pack (column block m of tile k, 8 channels, [128, 3136]):
  DVE  two-stage strided reduce            -> pooled sums [128, 64]
  PE   transpose -> fused [Wq|Wk] matmul   -> qkT [64, 128]
  ACT  bias-add via activation(Copy, bias) (fp32r out)
  PE   scores S = q@k.T [128, 128]; DVE +mask; ACT exp with accum_out -> Z
       (no max-subtraction: |S| < 0.1 by construction, exp cannot overflow)
  DVE  reciprocal(Z); PE transpose(exp) -> attT (fp32r)
  PE   att @ v in 7 N=448 chunks; 1/Z applied during PSUM evacuation
       (DVE tensor_scalar_mul / ACT activation scale) into the out tile.
x and all PE operands are declared float32r (same bits as fp32) so the DMA
lands matmul-ready data and no rounding copy is needed.
1/49 (pool mean) and 1/sqrt(16) (score scale) are folded into Wq/bq/Wk.
"""

import numpy as np

B, T, C, H, W = 8, 16, 64, 56, 56
DS = 8
DIN = DS * DS  # 64
DOUT = 32
HW = H * W  # 3136
CG = 8  # channels per compute pack
P = CG * T  # 128 partitions
NCH = 7  # output free-dim chunks per pack
CHN = HW // NCH  # 448
N_CORES = 8
MASK_NEG = -30.0
NTILE = 8  # DMA tiles; one 8-channel pack each (12544B descriptors)


def _build_nc():
    import concourse.bacc as bacc
    import concourse.tile as tile
    from concourse import mybir
    from contextlib import ExitStack

    f32 = mybir.dt.float32
    f32r = mybir.dt.float32r
    nc = bacc.Bacc(trn_type="TRN2", num_swdge_queues=2)

    # float32r has identical bits/numpy dtype to float32; declaring the DRAM
    # input as f32r lets the DMA land PE-ready tiles without a rounding copy.
    x_h = nc.dram_tensor("x", [T, C, H, W], f32r, kind="ExternalInput")
    # consts packed into ONE [128, 450] array (single DMA, 1800B rows):
    #   cols 0:128 additive mask; cols 129:193 = [Wq_eff | Wk_eff] (rows 0:64)
    #   with the bias in row 64; cols 194:322 all-ones (the qk matmul's
    #   bias row); cols 322:450 identity (PE transposes) — shipping identity
    #   and ones via DMA avoids float32r Memset (invalid ISA value type).
    cn_h = nc.dram_tensor("consts", [P, 450], f32r, kind="ExternalInput")
    out_h = nc.dram_tensor("out", [T, C, H, W], f32, kind="ExternalOutput")

    X = mybir.AxisListType.X
    Exp = mybir.ActivationFunctionType.Exp
    Copy = mybir.ActivationFunctionType.Copy

    with ExitStack() as ctx:
        tc = ctx.enter_context(tile.TileContext(nc))
        singles = ctx.enter_context(tc.tile_pool(name="singles", bufs=1))
        # one shared pool for the 8 input + 8 output tiles: 14 slots — only
        # out tiles 6,7 recycle (WAR) the slots of input tiles 0,1, whose
        # readers finish long before; fewer reused slots = fewer WAR waits
        # coupling late outputs to early packs' chunk matmuls.
        vpool = ctx.enter_context(tc.tile_pool(name="vpool", bufs=14))
        small = ctx.enter_context(tc.tile_pool(name="small", bufs=2))
        attpool = ctx.enter_context(tc.tile_pool(name="attpool", bufs=3))
        psA = ctx.enter_context(tc.tile_pool(name="psA", bufs=1, space="PSUM"))
        psB = ctx.enter_context(tc.tile_pool(name="psB", bufs=4, space="PSUM"))

        # consts ride the (otherwise output-only) gpsimd queue
        consts = singles.tile([P, 450], f32r)
        nc.gpsimd.dma_start(out=consts, in_=cn_h[:])
        mask = consts[:, 0:128]
        # rows 0:64 = [Wq_eff | Wk_eff]; row 64 = [bq_eff; bk_eff] — the bias
        # rides the matmul via a ones-row appended to pooledT
        wqkb = consts[0 : DIN + 1, 129:193]
        ones_row = consts[DIN : DIN + 1, 194:322]
        ident = consts[:, 322:450]

        x_ap = x_h[:]
        out_ap = out_h[:]

        # ---- input tiles: a single queue caps at ~220 GB/s, so inputs
        # alternate the sync/scalar HWDGE queues (pairs arrive every ~12us,
        # preserving rough order at the full ~260 GB/s engine rate) ----
        v_tiles = []
        for k in range(NTILE):
            c0 = CG * k
            v = vpool.tile([P, HW], f32r, tag="vio")
            src = x_ap[:, c0 : c0 + CG, :, :].rearrange("t c h w -> t c (h w)")
            eng = nc.sync if k % 2 == 0 else nc.scalar
            eng.dma_start(out=v[:], in_=src)
            v_tiles.append(v)

        stage2 = []  # (pack, v_pack, attT, rinv)

        def emit_stage1(p):
            v_pack = v_tiles[p][:]

            # ---- adaptive avg pool (sums; /49 folded into weights) ----
            tmp = small.tile([P, H, DS], f32, tag="tmp")
            nc.vector.reduce_sum(
                out=tmp[:],
                in_=v_pack.rearrange("p (h j vv) -> p h j vv", h=H, j=DS, vv=7),
                axis=X,
            )
            pooled = small.tile([P, DS, DS], f32r, tag="pooled")
            # f32r storage is bit-identical to f32 — no precision loss here
            with nc.allow_low_precision(reason="float32r bits == float32"):
                nc.vector.reduce_sum(
                    out=pooled[:],
                    in_=tmp[:].rearrange("p (i u) j -> p i j u", i=DS, u=7),
                    axis=X,
                )

            # ---- pooled^T so q/k matmuls contract over d_in ----
            pooledT_ps = psA.tile([DIN, P], f32r, tag="pooledT_ps")
            nc.tensor.transpose(
                pooledT_ps, pooled[:].rearrange("p i j -> p (i j)"), ident
            )
            pooledT = small.tile([DIN + 1, P], f32r, tag="pooledT")
            nc.scalar.copy(pooledT[0:DIN], pooledT_ps)
            nc.scalar.copy(pooledT[DIN : DIN + 1, :], ones_row)

            # ---- fused [q; k]^T [64, 128]; bias via the ones-row ----
            qk_ps = psA.tile([2 * DOUT, P], f32, tag="qk_ps")
            nc.tensor.matmul(qk_ps, lhsT=wqkb, rhs=pooledT[:], start=True, stop=True)
            qT = small.tile([DOUT, P], f32r, tag="qT")
            nc.scalar.copy(qT, qk_ps[0:DOUT])
            kT = small.tile([DOUT, P], f32r, tag="kT")
            nc.scalar.copy(kT, qk_ps[DOUT : 2 * DOUT])

            # ---- scores S[t, s] = q_t . k_s, full 128x128 cross ----
            sc_ps = psA.tile([P, P], f32, tag="sc_ps")
            nc.tensor.matmul(sc_ps, lhsT=qT[:], rhs=kT[:], start=True, stop=True)
            scm = small.tile([P, P], f32, tag="scm")
            nc.vector.tensor_add(out=scm, in0=sc_ps, in1=mask)

            # ---- exp (scores are tiny: skip max-subtract); Z via accum ----
            e = small.tile([P, P], f32r, tag="e")
            zs = small.tile([P, 1], f32, tag="zs")
            nc.scalar.activation(out=e, in_=scm, func=Exp, scale=1.0, accum_out=zs)
            rinv = small.tile([P, 1], f32, tag="rinv")
            nc.vector.reciprocal(rinv, zs)

            # ---- att^T (unnormalized) becomes the stationary operand ----
            eT_ps = psA.tile([P, P], f32r, tag="eT_ps")
            nc.tensor.transpose(eT_ps, e[:], ident)
            attT = attpool.tile([P, P], f32r, tag="attT")
            nc.scalar.copy(attT, eT_ps)
            stage2.append((p, v_pack, attT, rinv))

        def emit_stage2(p, v_pack, attT, rinv):
            o = vpool.tile([P, HW], f32, tag="vio", name=f"otile{p}")
            for ch in range(NCH):
                sl = slice(ch * CHN, (ch + 1) * CHN)
                ops = psB.tile([P, CHN], f32, tag="ochunk")
                nc.tensor.matmul(
                    ops, lhsT=attT[:], rhs=v_pack[:, sl], start=True, stop=True
                )
                # 1/Z applied during PSUM->SBUF evacuation (split DVE/ACT;
                # ACT does chunk 6 last so even packs' out-DMA needs no
                # cross-engine wait on its own issuing engine)
                if ch in (2, 5):
                    nc.vector.tensor_scalar_mul(out=o[:, sl], in0=ops, scalar1=rinv)
                else:
                    nc.scalar.activation(out=o[:, sl], in_=ops, func=Copy, scale=rinv)
            c0 = CG * p
            dst = out_ap[:, c0 : c0 + CG, :, :].rearrange("t c h w -> t c (h w)")
            # issue engines (gpsimd/sync) are idle — no head-of-line risk to
            # ACT/DVE compute; the sync QUEUE's input backlog delays odd
            # outputs slightly, which the even/gpsimd stream covers
            eng = nc.gpsimd if p % 2 == 0 else nc.sync
            eng.dma_start(out=dst, in_=o[:])

        for p in range(NTILE):
            emit_stage1(p)
            if p >= 1:
                emit_stage2(*stage2[p - 1])
        emit_stage2(*stage2[NTILE - 1])

    nc.compile()
    return nc


def _host_consts(Wq, bq, Wk, bk):
    # fold pool-mean 1/49 into both weight mats; fold score 1/sqrt(t)=1/4
    # into the q side (weights AND bias)
    wq_eff = (Wq / (49.0 * 4.0)).astype(np.float32)
    bq_eff = (bq / 4.0).astype(np.float32)
    wk_eff = (Wk / 49.0).astype(np.float32)
    bk_eff = bk.astype(np.float32)
    # partition i = t*8 + j; attention pairs (i, i') share a channel iff
    # i%8 == i'%8 (for every tile/block, channel = 16k + 2j + m)
    idx = np.arange(P)
    same_c = np.equal.outer(idx % CG, idx % CG)
    mask = np.where(same_c, 0.0, MASK_NEG).astype(np.float32)
    consts = np.zeros((P, 450), dtype=np.float32)
    consts[:, 0:128] = mask
    consts[0:DIN, 129:161] = wq_eff
    consts[0:DIN, 161:193] = wk_eff
    consts[DIN, 129:161] = bq_eff
    consts[DIN, 161:193] = bk_eff
    consts[:, 194:322] = 1.0
    consts[:, 322:450] = np.eye(P, dtype=np.float32)
    return consts


def kernel(x, Wq, bq, Wk, bk):
    from concourse.bass_utils import run_bass_kernel_spmd

    x = np.ascontiguousarray(x, dtype=np.float32)
    consts = _host_consts(Wq, bq, Wk, bk)

    nc = _build_nc()
    in_maps = [{"x": x[i], "consts": consts} for i in range(N_CORES)]
    res = run_bass_kernel_spmd(nc, in_maps, core_ids=list(range(N_CORES)))
    global LAST_RUN
    LAST_RUN = res
    out = np.stack([r["out"] for r in res.results], axis=0)
    return out


LAST_RUN = None


# revision 16
# speedup vs baseline: 1.0071x; 1.0071x over previous
"""ChannelTimeAttention Trainium2 kernel.

Reference computation (per (b, c) pair, all independent):
    pooled = AdaptiveAvgPool(x[b, :, c]) -> [t, 8*8]      (7x7 block means)
    q = pooled @ Wq + bq ; k = pooled @ Wk + bk           [t, 32]
    att = softmax(q @ k.T / sqrt(t))                      [t, t]
    out[b, :, c] = att @ x[b, :, c].reshape(t, h*w)

Sharding: data-parallel over b — one batch element per NeuronCore (8 cores).

DMA design (trace-measured): the 16 per-core DMA engines process
descriptors serially at ~16.4 GB/s each with 12544B per-partition-row
descriptors (~262 GB/s aggregate; bigger 25KB descriptors measured SLOWER
at 14.9 GB/s, so one channel per partition row is optimal).  A single
HWDGE queue sustains only ~220 GB/s, so the 8 input tiles [128, 3136]
(partition t*8+c, t-major for near-sequential DRAM walks) alternate the
sync/scalar queues — pairs arrive every ~12us at the full engine rate in
rough pack order.  Outputs alternate gpsimd/sync; their dma_starts are
issued by the otherwise-idle gpsimd/sync engines so sem-waits never
head-of-line block ACT/DVE compute.  Inputs+outputs share one 14-slot
SBUF pool (out tiles 6,7 recycle input slots 0,1 via WAR semaphores).

Compute per pack (8 channels, [128, 3136]):
  DVE  two-stage strided reduce            -> pooled sums [128, 64]
  PE   transpose -> fused [Wq|Wk] matmul   -> qkT [64, 128]
  ACT  bias-add via activation(Copy, bias) (fp32r out)
  PE   scores S = q@k.T [128, 128]; DVE +mask; ACT exp with accum_out -> Z
       (no max-subtraction: |S| < 0.1 by construction, exp cannot overflow)
  DVE  reciprocal(Z); PE transpose(exp) -> attT (fp32r)
  PE   att @ v in 7 N=448 chunks; 1/Z applied during PSUM evacuation
       (DVE tensor_scalar_mul / ACT activation scale) into the out tile.
x and all PE operands are declared float32r (same bits as fp32) so the DMA
lands matmul-ready data and no rounding copy is needed.
1/49 (pool mean) and 1/sqrt(16) (score scale) are folded into Wq/bq/Wk.
"""

import numpy as np

B, T, C, H, W = 8, 16, 64, 56, 56
DS = 8
DIN = DS * DS  # 64
DOUT = 32
HW = H * W  # 3136
CG = 8  # channels per compute pack
P = CG * T  # 128 partitions
NCH = 7  # output free-dim chunks per pack
CHN = HW // NCH  # 448
N_CORES = 8
MASK_NEG = -30.0
NTILE = 8  # DMA tiles; one 8-channel pack each (12544B descriptors)


def _build_nc():
    import concourse.bacc as bacc
    import concourse.tile as tile
    from concourse import mybir
    from contextlib import ExitStack

    f32 = mybir.dt.float32
    f32r = mybir.dt.float32r
    nc = bacc.Bacc(trn_type="TRN2", num_swdge_queues=2)

    # float32r has identical bits/numpy dtype to float32; declaring the DRAM
    # input as f32r lets the DMA land PE-ready tiles without a rounding copy.
    x_h = nc.dram_tensor("x", [T, C, H, W], f32r, kind="ExternalInput")
    # consts packed into ONE [128, 450] array (single DMA, 1800B rows):
    #   cols 0:128 additive mask; cols 129:193 = [Wq_eff | Wk_eff] (rows 0:64)
    #   with the bias in row 64; cols 194:322 all-ones (the qk matmul's
    #   bias row); cols 322:450 identity (PE transposes) — shipping identity
    #   and ones via DMA avoids float32r Memset (invalid ISA value type).
    cn_h = nc.dram_tensor("consts", [P, 450], f32r, kind="ExternalInput")
    out_h = nc.dram_tensor("out", [T, C, H, W], f32, kind="ExternalOutput")

    X = mybir.AxisListType.X
    Exp = mybir.ActivationFunctionType.Exp
    Copy = mybir.ActivationFunctionType.Copy

    with ExitStack() as ctx:
        tc = ctx.enter_context(tile.TileContext(nc))
        singles = ctx.enter_context(tc.tile_pool(name="singles", bufs=1))
        # one shared pool for the 8 input + 8 output tiles: 14 slots — only
        # out tiles 6,7 recycle (WAR) the slots of input tiles 0,1, whose
        # readers finish long before; fewer reused slots = fewer WAR waits
        # coupling late outputs to early packs' chunk matmuls.
        vpool = ctx.enter_context(tc.tile_pool(name="vpool", bufs=14))
        small = ctx.enter_context(tc.tile_pool(name="small", bufs=3))
        attpool = ctx.enter_context(tc.tile_pool(name="attpool", bufs=4))
        psA = ctx.enter_context(tc.tile_pool(name="psA", bufs=1, space="PSUM"))
        psB = ctx.enter_context(tc.tile_pool(name="psB", bufs=4, space="PSUM"))

        # consts ride the (otherwise output-only) gpsimd queue
        consts = singles.tile([P, 450], f32r)
        nc.gpsimd.dma_start(out=consts, in_=cn_h[:])
        mask = consts[:, 0:128]
        # rows 0:64 = [Wq_eff | Wk_eff]; row 64 = [bq_eff; bk_eff] — the bias
        # rides the matmul via a ones-row appended to pooledT
        wqkb = consts[0 : DIN + 1, 129:193]
        ones_row = consts[DIN : DIN + 1, 194:322]
        ident = consts[:, 322:450]

        x_ap = x_h[:]
        out_ap = out_h[:]

        # ---- input tiles: a single queue caps at ~220 GB/s, so inputs
        # alternate the sync/scalar HWDGE queues (pairs arrive every ~12us,
        # preserving rough order at the full ~260 GB/s engine rate) ----
        v_tiles = []
        for k in range(NTILE):
            c0 = CG * k
            v = vpool.tile([P, HW], f32r, tag="vio")
            src = x_ap[:, c0 : c0 + CG, :, :].rearrange("t c h w -> t c (h w)")
            eng = nc.sync if k % 2 == 0 else nc.scalar
            eng.dma_start(out=v[:], in_=src)
            v_tiles.append(v)

        stage2 = []  # (pack, v_pack, attT, rinv)

        def emit_stage1(p):
            v_pack = v_tiles[p][:]

            # ---- adaptive avg pool (sums; /49 folded into weights) ----
            tmp = small.tile([P, H, DS], f32, tag="tmp")
            nc.vector.reduce_sum(
                out=tmp[:],
                in_=v_pack.rearrange("p (h j vv) -> p h j vv", h=H, j=DS, vv=7),
                axis=X,
            )
            pooled = small.tile([P, DS, DS], f32r, tag="pooled")
            # f32r storage is bit-identical to f32 — no precision loss here
            with nc.allow_low_precision(reason="float32r bits == float32"):
                nc.vector.reduce_sum(
                    out=pooled[:],
                    in_=tmp[:].rearrange("p (i u) j -> p i j u", i=DS, u=7),
                    axis=X,
                )

            # ---- pooled^T so q/k matmuls contract over d_in ----
            pooledT_ps = psA.tile([DIN, P], f32r, tag="pooledT_ps")
            nc.tensor.transpose(
                pooledT_ps, pooled[:].rearrange("p i j -> p (i j)"), ident
            )
            pooledT = small.tile([DIN + 1, P], f32r, tag="pooledT")
            nc.scalar.copy(pooledT[0:DIN], pooledT_ps)
            nc.scalar.copy(pooledT[DIN : DIN + 1, :], ones_row)

            # ---- fused [q; k]^T [64, 128]; bias via the ones-row ----
            qk_ps = psA.tile([2 * DOUT, P], f32, tag="qk_ps")
            nc.tensor.matmul(qk_ps, lhsT=wqkb, rhs=pooledT[:], start=True, stop=True)
            qT = small.tile([DOUT, P], f32r, tag="qT")
            nc.scalar.copy(qT, qk_ps[0:DOUT])
            kT = small.tile([DOUT, P], f32r, tag="kT")
            nc.scalar.copy(kT, qk_ps[DOUT : 2 * DOUT])

            # ---- scores S[t, s] = q_t . k_s, full 128x128 cross ----
            sc_ps = psA.tile([P, P], f32, tag="sc_ps")
            nc.tensor.matmul(sc_ps, lhsT=qT[:], rhs=kT[:], start=True, stop=True)
            scm = small.tile([P, P], f32, tag="scm")
            nc.vector.tensor_add(out=scm, in0=sc_ps, in1=mask)

            # ---- exp (scores are tiny: skip max-subtract); Z via accum ----
            e = small.tile([P, P], f32r, tag="e")
            zs = small.tile([P, 1], f32, tag="zs")
            nc.scalar.activation(out=e, in_=scm, func=Exp, scale=1.0, accum_out=zs)
            rinv = small.tile([P, 1], f32, tag="rinv")
            nc.vector.reciprocal(rinv, zs)

            # ---- att^T (unnormalized) becomes the stationary operand ----
            eT_ps = psA.tile([P, P], f32r, tag="eT_ps")
            nc.tensor.transpose(eT_ps, e[:], ident)
            attT = attpool.tile([P, P], f32r, tag="attT")
            nc.scalar.copy(attT, eT_ps)
            stage2.append((p, v_pack, attT, rinv))

        def emit_stage2(p, v_pack, attT, rinv):
            o = vpool.tile([P, HW], f32, tag="vio", name=f"otile{p}")
            for ch in range(NCH):
                sl = slice(ch * CHN, (ch + 1) * CHN)
                ops = psB.tile([P, CHN], f32, tag="ochunk")
                nc.tensor.matmul(
                    ops, lhsT=attT[:], rhs=v_pack[:, sl], start=True, stop=True
                )
                # 1/Z applied during PSUM->SBUF evacuation (split DVE/ACT;
                # ACT does chunk 6 last so even packs' out-DMA needs no
                # cross-engine wait on its own issuing engine)
                if ch in (2, 5):
                    nc.vector.tensor_scalar_mul(out=o[:, sl], in0=ops, scalar1=rinv)
                else:
                    nc.scalar.activation(out=o[:, sl], in_=ops, func=Copy, scale=rinv)
            c0 = CG * p
            dst = out_ap[:, c0 : c0 + CG, :, :].rearrange("t c h w -> t c (h w)")
            # issue engines (gpsimd/sync) are idle — no head-of-line risk to
            # ACT/DVE compute; the sync QUEUE's input backlog delays odd
            # outputs slightly, which the even/gpsimd stream covers
            eng = nc.gpsimd if p % 2 == 0 else nc.sync
            eng.dma_start(out=dst, in_=o[:])

        for p in range(NTILE):
            emit_stage1(p)
            if p >= 1:
                emit_stage2(*stage2[p - 1])
        emit_stage2(*stage2[NTILE - 1])

    nc.compile()
    return nc


def _host_consts(Wq, bq, Wk, bk):
    # fold pool-mean 1/49 into both weight mats; fold score 1/sqrt(t)=1/4
    # into the q side (weights AND bias)
    wq_eff = (Wq / (49.0 * 4.0)).astype(np.float32)
    bq_eff = (bq / 4.0).astype(np.float32)
    wk_eff = (Wk / 49.0).astype(np.float32)
    bk_eff = bk.astype(np.float32)
    # partition i = t*8 + j; attention pairs (i, i') share a channel iff
    # i%8 == i'%8 (for every tile/block, channel = 16k + 2j + m)
    idx = np.arange(P)
    same_c = np.equal.outer(idx % CG, idx % CG)
    mask = np.where(same_c, 0.0, MASK_NEG).astype(np.float32)
    consts = np.zeros((P, 450), dtype=np.float32)
    consts[:, 0:128] = mask
    consts[0:DIN, 129:161] = wq_eff
    consts[0:DIN, 161:193] = wk_eff
    consts[DIN, 129:161] = bq_eff
    consts[DIN, 161:193] = bk_eff
    consts[:, 194:322] = 1.0
    consts[:, 322:450] = np.eye(P, dtype=np.float32)
    return consts


def kernel(x, Wq, bq, Wk, bk):
    from concourse.bass_utils import run_bass_kernel_spmd

    x = np.ascontiguousarray(x, dtype=np.float32)
    consts = _host_consts(Wq, bq, Wk, bk)

    nc = _build_nc()
    in_maps = [{"x": x[i], "consts": consts} for i in range(N_CORES)]
    res = run_bass_kernel_spmd(nc, in_maps, core_ids=list(range(N_CORES)))
    global LAST_RUN
    LAST_RUN = res
    out = np.stack([r["out"] for r in res.results], axis=0)
    return out


LAST_RUN = None
